# revision 10
# baseline (speedup 1.0000x reference)
"""Trainium2 Bass kernel for nn_Concat_84653805404637.

Problem: x [40, 256, 256] f32.  supports = x[:10], queries = x[10:40].
out[i*10 + j] = concat(supports[j], queries[i], axis=-1)  -> [300, 256, 512].

Pure data movement (memory regime).  Strategy:
  - SPMD over 8 cores: every core runs the identical program producing
    out_c [40, 256, 512] = 4 queries x 10 supports.
  - Host hands each core the shared 10 support rows plus its 4 query rows
    (cores 6/7 overlap queries so 8*4 >= 30; host drops the overlap).
  - On-device v1: inputs staged once in SBUF (partition = t within a
    128-row t-half), then large HWDGE DMAs write the contiguous output
    slab, broadcasting query rows across pair slots via step-0 APs.
    Stores are split across both HWDGE rings (sync + scalar).
"""

import numpy as np

import concourse.bass as bass
import concourse.mybir as mybir
from concourse.bass_utils import run_bass_kernel_spmd

N_CORES = 8
NSUP = 10  # support rows actually used by the reference (x0 block + x1 block)
NQ = 4  # queries per core (8*4 = 32 slots for 30 queries; overlap discarded)
T = 256
F = 256
NQ_TOTAL = 30
TH = T // 2  # 128-row t-half -> SBUF partition dim

# core c processes queries QSTART[c] : QSTART[c]+4
QSTART = [0, 4, 8, 12, 16, 20, 24, 26]
# host keeps local pair rows KEEP[c] from each core's [40, ...] output.
# cores 0..6 cover pairs 0..280; core 7 covers 260..300, keep its 280..300.
KEEP = [(0, 40)] * 7 + [(20, 40)]

_CACHE: dict = {}


def _build_kernel_v0() -> bass.Bass:
    """DRAM->DRAM broadcast DMAs, no SBUF staging (baseline)."""
    nc = bass.Bass("TRN2", target_bir_lowering=False)
    sup = nc.dram_tensor("sup", [NSUP, T, F], mybir.dt.float32, kind="ExternalInput")
    qry = nc.dram_tensor("qry", [NQ, T, F], mybir.dt.float32, kind="ExternalInput")
    out = nc.dram_tensor(
        "out", [NQ * NSUP, T, 2 * F], mybir.dt.float32, kind="ExternalOutput"
    )

    with nc.semaphore("dma_sem") as dma_sem, nc.Block() as block:

        @block.sync
        def _(sync):
            n = 0
            for il in range(NQ):
                sync.dma_start(
                    out[il * NSUP : (il + 1) * NSUP, :, 0:F],
                    sup[:, :, :],
                ).then_inc(dma_sem, 16)
                n += 1
                sync.dma_start(
                    out[il * NSUP : (il + 1) * NSUP, :, F : 2 * F],
                    qry[il][None, :, :].broadcast_to([NSUP, T, F]),
                ).then_inc(dma_sem, 16)
                n += 1
            sync.wait_ge(dma_sem, n * 16)

    return nc


def _build_kernel_v1() -> bass.Bass:
    """SBUF-staged: load sup/qry once, fan out with broadcast store DMAs."""
    nc = bass.Bass("TRN2", target_bir_lowering=False)
    sup = nc.dram_tensor("sup", [NSUP, T, F], mybir.dt.float32, kind="ExternalInput")
    qry = nc.dram_tensor("qry", [NQ, T, F], mybir.dt.float32, kind="ExternalInput")
    out = nc.dram_tensor(
        "out", [NQ * NSUP, T, 2 * F], mybir.dt.float32, kind="ExternalOutput"
    )

    f32 = mybir.dt.float32
    with (
        nc.sbuf_tensor("supA", [128, NSUP * F], f32) as supA,
        nc.sbuf_tensor("supB", [128, NSUP * F], f32) as supB,
        nc.sbuf_tensor("qryA", [128, NQ * F], f32) as qryA,
        nc.sbuf_tensor("qryB", [128, NQ * F], f32) as qryB,
        nc.semaphore("ls_a") as ls_a,
        nc.semaphore("ls_b") as ls_b,
        nc.semaphore("lq_a") as lq_a,
        nc.semaphore("lq_b") as lq_b,
        nc.semaphore("done") as done,
        nc.Block() as block,
    ):
        sup_tiles = {0: supA, 1: supB}
        qry_tiles = {0: qryA, 1: qryB}
        sup_sems = {0: ls_a, 1: ls_b}
        qry_sems = {0: lq_a, 1: lq_b}

        # SBUF APs must keep the partition dim first; permute the DRAM dst
        # to the same (t, pair, f) iteration order instead.
        def sup_store(eng, il, h):
            # out[il*10:(il+1)*10, h*128:(h+1)*128, 0:F] = sup rows
            dst = out[il * NSUP : (il + 1) * NSUP, h * TH : (h + 1) * TH, 0:F]
            src = sup_tiles[h].ap().rearrange("p (j f) -> p j f", f=F)
            eng.dma_start(dst.transpose([1, 0, 2]), src).then_inc(done, 16)

        def qry_store(eng, il, h):
            dst = out[il * NSUP : (il + 1) * NSUP, h * TH : (h + 1) * TH, F : 2 * F]
            src = (
                qry_tiles[h]
                .ap()[:, il * F : (il + 1) * F]
                .unsqueeze(1)
                .broadcast_to([TH, NSUP, F])
            )
            eng.dma_start(dst.transpose([1, 0, 2]), src).then_inc(done, 16)

        @block.sync
        def _(sync):
            # loads: sup t-halves
            for h in (0, 1):
                sync.dma_start(
                    sup_tiles[h].ap(),
                    sup[:, h * TH : (h + 1) * TH, :].transpose([1, 0, 2]),
                ).then_inc(sup_sems[h], 16)
            for h in (0, 1):
                sync.wait_ge(sup_sems[h], 16)
                for il in range(NQ):
                    sup_store(sync, il, h)
            # everything must have landed before the NEFF may finish
            sync.wait_ge(done, 16 * 16)

        @block.scalar
        def _(scalar):
            for h in (0, 1):
                scalar.dma_start(
                    qry_tiles[h].ap(),
                    qry[:, h * TH : (h + 1) * TH, :].transpose([1, 0, 2]),
                ).then_inc(qry_sems[h], 16)
            for h in (0, 1):
                scalar.wait_ge(qry_sems[h], 16)
                for il in range(NQ):
                    qry_store(scalar, il, h)

    return nc


def _build_kernel_v3() -> bass.Bass:
    """DVE-assembled output rows in SBUF -> 4 KB DMA descriptors.

    SBUF partition p = t // 2 (each partition holds t rows 2p, 2p+1), so an
    assembled pair row block is contiguous for 4 KB in both SBUF free dim
    and the output: descriptors are 4x bigger than the raw 1 KB (f-run)
    limit, lifting the per-SDMA-engine descriptor-rate ceiling.

    Per query block il (10 pairs): staging S[p, (q, t2, f2)] free = 10*2*512.
      sup half:  S[p, q*1024 + t2*512 + f]       = sup[q, 2p+t2, f]
      qry half:  S[p, q*1024 + t2*512 + 256 + f] = qry[il, 2p+t2, f]
    One store DMA per (block, t2-pair granularity merged): dst flat block
    out[il*10:(il+1)*10] with AP [[1024,128],[131072,10],[1,1024]].
    """
    nc = bass.Bass("TRN2", target_bir_lowering=False)
    sup = nc.dram_tensor("sup", [NSUP, T, F], mybir.dt.float32, kind="ExternalInput")
    qry = nc.dram_tensor("qry", [NQ, T, F], mybir.dt.float32, kind="ExternalInput")
    out = nc.dram_tensor(
        "out", [NQ * NSUP, T, 2 * F], mybir.dt.float32, kind="ExternalOutput"
    )

    f32 = mybir.dt.float32
    NBUF = 2  # staging double buffer
    with (
        nc.sbuf_tensor("supt", [128, NSUP * 2 * F], f32) as supt,
        nc.sbuf_tensor("qryt", [128, NQ * 2 * F], f32) as qryt,
        nc.sbuf_tensor("stg0", [128, NSUP * 2 * 2 * F], f32) as stg0,
        nc.sbuf_tensor("stg1", [128, NSUP * 2 * 2 * F], f32) as stg1,
        nc.semaphore("load_sup") as load_sup,
        nc.semaphore("load_qry") as load_qry,
        nc.semaphore("dve_done") as dve_done,
        nc.semaphore("store_even") as store_even,
        nc.semaphore("store_odd") as store_odd,
        nc.Block() as block,
    ):
        stg = [stg0, stg1]
        store_sem = [store_even, store_odd]
        SW = NSUP * 2 * 2 * F  # staging free width (20480)

        def store_block(eng, il):
            eng.wait_ge(dve_done, 4 * (il + 1))
            dst = bass.AP(
                out,
                il * NSUP * T * 2 * F,
                [[2 * 2 * F, 128], [T * 2 * F, NSUP], [1, 2 * 2 * F]],
            )
            src = bass.AP(
                stg[il % NBUF], 0, [[SW, 128], [2 * 2 * F, NSUP], [1, 2 * 2 * F]]
            )
            eng.dma_start(dst, src).then_inc(store_sem[il % 2], 16)

        @block.sync
        def _(sync):
            # loads: input tiles with partition = t//2, free = (row, t2, f)
            for t2 in (0, 1):
                dst = supt.ap().rearrange("p (j t2 f) -> p j t2 f", t2=2, f=F)[
                    :, :, t2, :
                ]
                src = bass.AP(sup, t2 * F, [[2 * F, 128], [T * F, NSUP], [1, F]])
                sync.dma_start(dst, src).then_inc(load_sup, 16)
            for t2 in (0, 1):
                dst = qryt.ap().rearrange("p (i t2 f) -> p i t2 f", t2=2, f=F)[
                    :, :, t2, :
                ]
                src = bass.AP(qry, t2 * F, [[2 * F, 128], [T * F, NQ], [1, F]])
                sync.dma_start(dst, src).then_inc(load_qry, 16)
            for il in range(0, NQ, 2):
                store_block(sync, il)
            sync.wait_ge(store_even, 16 * (NQ // 2))
            sync.wait_ge(store_odd, 16 * (NQ // 2))

        @block.scalar
        def _(scalar):
            for il in range(1, NQ, 2):
                store_block(scalar, il)

        @block.vector
        def _(vector):
            vector.wait_ge(load_sup, 32)
            vector.wait_ge(load_qry, 32)
            for il in range(NQ):
                if il >= NBUF:
                    # staging slot reuse: wait for store of block il-NBUF
                    vector.wait_ge(store_sem[il % 2], 16 * ((il - NBUF) // 2 + 1))
                s = stg[il % NBUF]
                for t2 in (0, 1):
                    # sup half: S[p, q*1024 + t2*512 + f] = supt[p, q*512 + t2*256 + f]
                    dst = bass.AP(s, t2 * 2 * F, [[SW, 128], [2 * 2 * F, NSUP], [1, F]])
                    src = bass.AP(supt, t2 * F, [[NSUP * 2 * F, 128], [2 * F, NSUP], [1, F]])
                    vector.tensor_copy(dst, src).then_inc(dve_done, 1)
                    # qry half: S[..., +256] = qryt[p, il*512 + t2*256 + f] bcast over q
                    dstq = bass.AP(s, t2 * 2 * F + F, [[SW, 128], [2 * 2 * F, NSUP], [1, F]])
                    srcq = bass.AP(qryt, il * 2 * F + t2 * F, [[NQ * 2 * F, 128], [0, NSUP], [1, F]])
                    vector.tensor_copy(dstq, srcq).then_inc(dve_done, 1)

    return nc


def _build_kernel_v4() -> bass.Bass:
    """v3 + finer pipeline: 8 half-blocks (5 pairs) over 4 staging buffers,
    stores round-robined over 3 DMA queues (sync, scalar HWDGE + gpsimd
    SWDGE), loads split across queues, sup halves assembled once per buffer.
    """
    nc = bass.Bass("TRN2", target_bir_lowering=False)
    sup = nc.dram_tensor("sup", [NSUP, T, F], mybir.dt.float32, kind="ExternalInput")
    qry = nc.dram_tensor("qry", [NQ, T, F], mybir.dt.float32, kind="ExternalInput")
    out = nc.dram_tensor(
        "out", [NQ * NSUP, T, 2 * F], mybir.dt.float32, kind="ExternalOutput"
    )

    f32 = mybir.dt.float32
    HP = 5  # pairs per half-block
    NHB = (NQ * NSUP) // HP  # 8 half-blocks
    NBUF = 4
    SW = HP * 2 * 2 * F  # staging free width (5120)
    SUPW = NSUP * 2 * F  # supt free width
    QRYW = NQ * 2 * F
    with (
        nc.sbuf_tensor("supt", [128, SUPW], f32) as supt,
        nc.sbuf_tensor("qryt", [128, QRYW], f32) as qryt,
        nc.sbuf_tensor("hb0", [128, SW], f32) as hb0,
        nc.sbuf_tensor("hb1", [128, SW], f32) as hb1,
        nc.sbuf_tensor("hb2", [128, SW], f32) as hb2,
        nc.sbuf_tensor("hb3", [128, SW], f32) as hb3,
        nc.semaphore("load_sup") as load_sup,
        nc.semaphore("load_qry") as load_qry,
        nc.semaphore("dve_done") as dve_done,
        nc.semaphore("st_h0") as st_h0,
        nc.semaphore("st_h1") as st_h1,
        nc.semaphore("st_h2") as st_h2,
        nc.semaphore("st_h3") as st_h3,
        nc.semaphore("st_h4") as st_h4,
        nc.semaphore("st_h5") as st_h5,
        nc.semaphore("st_h6") as st_h6,
        nc.semaphore("st_h7") as st_h7,
        nc.Block() as block,
    ):
        hb = [hb0, hb1, hb2, hb3]
        # one sem per half-block store (SWDGE sems must start at 0)
        st_sem = [st_h0, st_h1, st_h2, st_h3, st_h4, st_h5, st_h6, st_h7]

        def load_sup_dma(eng, t2):
            dst = supt.ap().rearrange("p (j t2 f) -> p j t2 f", t2=2, f=F)[:, :, t2, :]
            src = bass.AP(sup, t2 * F, [[2 * F, 128], [T * F, NSUP], [1, F]])
            eng.dma_start(dst, src).then_inc(load_sup, 16)

        def load_qry_dma(eng, t2):
            dst = qryt.ap().rearrange("p (i t2 f) -> p i t2 f", t2=2, f=F)[:, :, t2, :]
            src = bass.AP(qry, t2 * F, [[2 * F, 128], [T * F, NQ], [1, F]])
            eng.dma_start(dst, src).then_inc(load_qry, 16)

        def store_half(eng, h):
            # store half-block h: wait for its dve copies (8 sup + 2*(h+1))
            eng.wait_ge(dve_done, 8 + 2 * (h + 1))
            dst = bass.AP(
                out,
                h * HP * T * 2 * F,
                [[2 * 2 * F, 128], [T * 2 * F, HP], [1, 2 * 2 * F]],
            )
            src = bass.AP(hb[h % NBUF], 0, [[SW, 128], [2 * 2 * F, HP], [1, 2 * 2 * F]])
            eng.dma_start(dst, src).then_inc(st_sem[h], 16)

        @block.sync
        def _(sync):
            load_sup_dma(sync, 0)
            for h in range(0, NHB, 3):
                store_half(sync, h)
            for h in range(NHB):
                sync.wait_ge(st_sem[h], 16)

        @block.scalar
        def _(scalar):
            load_sup_dma(scalar, 1)
            for h in range(1, NHB, 3):
                store_half(scalar, h)

        @block.gpsimd
        def _(gpsimd):
            load_qry_dma(gpsimd, 0)
            load_qry_dma(gpsimd, 1)
            for h in range(2, NHB, 3):
                store_half(gpsimd, h)

        @block.vector
        def _(vector):
            # sup halves, once per buffer: buffer k serves half-blocks
            # h = k (mod 4), whose j-range is (k % 2) * 5
            vector.wait_ge(load_sup, 32)
            for k in range(NBUF):
                j0 = (k % 2) * HP
                for t2 in (0, 1):
                    dst = bass.AP(hb[k], t2 * 2 * F, [[SW, 128], [2 * 2 * F, HP], [1, F]])
                    src = bass.AP(
                        supt, j0 * 2 * F + t2 * F, [[SUPW, 128], [2 * F, HP], [1, F]]
                    )
                    vector.tensor_copy(dst, src).then_inc(dve_done, 1)
            # qry halves per half-block
            vector.wait_ge(load_qry, 32)
            for h in range(NHB):
                if h >= NBUF:
                    # buffer h%NBUF freed once its previous store completed
                    vector.wait_ge(st_sem[h - NBUF], 16)
                il = h // 2
                for t2 in (0, 1):
                    dst = bass.AP(
                        hb[h % NBUF], t2 * 2 * F + F, [[SW, 128], [2 * 2 * F, HP], [1, F]]
                    )
                    src = bass.AP(
                        qryt, il * 2 * F + t2 * F, [[QRYW, 128], [0, HP], [1, F]]
                    )
                    vector.tensor_copy(dst, src).then_inc(dve_done, 1)

    return nc


def _get_nc() -> bass.Bass:
    if "nc" not in _CACHE:
        _CACHE["nc"] = _build_kernel_v4()
    return _CACHE["nc"]


def kernel(x: np.ndarray) -> np.ndarray:
    x = np.asarray(x, dtype=np.float32)
    sup = np.ascontiguousarray(x[:NSUP])  # [10, 256, 256]
    queries = np.ascontiguousarray(x[10:])  # [30, 256, 256]

    in_maps = []
    for c in range(N_CORES):
        q0 = QSTART[c]
        in_maps.append(
            {
                "sup": sup,
                "qry": np.ascontiguousarray(queries[q0 : q0 + NQ]),
            }
        )

    nc = _get_nc()
    res = run_bass_kernel_spmd(nc, in_maps, core_ids=list(range(N_CORES)))

    parts = []
    for c in range(N_CORES):
        lo, hi = KEEP[c]
        parts.append(res.results[c]["out"][lo:hi])
    full = np.concatenate(parts, axis=0)
    assert full.shape == (NQ_TOTAL * NSUP, T, 2 * F)
    return full


# revision 12
# speedup vs baseline: 1.0137x; 1.0137x over previous
"""Trainium2 Bass kernel for nn_Concat_84653805404637.

Problem: x [40, 256, 256] f32.  supports = x[:10], queries = x[10:40].
out[i*10 + j] = concat(supports[j], queries[i], axis=-1)  -> [300, 256, 512].

Pure data movement (memory regime).  Strategy:
  - SPMD over 8 cores: every core runs the identical program producing
    out_c [40, 256, 512] = 4 queries x 10 supports.
  - Host hands each core the shared 10 support rows plus its 4 query rows
    (cores 6/7 overlap queries so 8*4 >= 30; host drops the overlap).
  - On-device v1: inputs staged once in SBUF (partition = t within a
    128-row t-half), then large HWDGE DMAs write the contiguous output
    slab, broadcasting query rows across pair slots via step-0 APs.
    Stores are split across both HWDGE rings (sync + scalar).
"""

import numpy as np

import concourse.bass as bass
import concourse.mybir as mybir
from concourse.bass_utils import run_bass_kernel_spmd

N_CORES = 8
NSUP = 10  # support rows actually used by the reference (x0 block + x1 block)
NQ = 4  # queries per core (8*4 = 32 slots for 30 queries; overlap discarded)
T = 256
F = 256
NQ_TOTAL = 30
TH = T // 2  # 128-row t-half -> SBUF partition dim

# core c processes queries QSTART[c] : QSTART[c]+4
QSTART = [0, 4, 8, 12, 16, 20, 24, 26]
# host keeps local pair rows KEEP[c] from each core's [40, ...] output.
# cores 0..6 cover pairs 0..280; core 7 covers 260..300, keep its 280..300.
KEEP = [(0, 40)] * 7 + [(20, 40)]

_CACHE: dict = {}


def _build_kernel_v0() -> bass.Bass:
    """DRAM->DRAM broadcast DMAs, no SBUF staging (baseline)."""
    nc = bass.Bass("TRN2", target_bir_lowering=False)
    sup = nc.dram_tensor("sup", [NSUP, T, F], mybir.dt.float32, kind="ExternalInput")
    qry = nc.dram_tensor("qry", [NQ, T, F], mybir.dt.float32, kind="ExternalInput")
    out = nc.dram_tensor(
        "out", [NQ * NSUP, T, 2 * F], mybir.dt.float32, kind="ExternalOutput"
    )

    with nc.semaphore("dma_sem") as dma_sem, nc.Block() as block:

        @block.sync
        def _(sync):
            n = 0
            for il in range(NQ):
                sync.dma_start(
                    out[il * NSUP : (il + 1) * NSUP, :, 0:F],
                    sup[:, :, :],
                ).then_inc(dma_sem, 16)
                n += 1
                sync.dma_start(
                    out[il * NSUP : (il + 1) * NSUP, :, F : 2 * F],
                    qry[il][None, :, :].broadcast_to([NSUP, T, F]),
                ).then_inc(dma_sem, 16)
                n += 1
            sync.wait_ge(dma_sem, n * 16)

    return nc


def _build_kernel_v1() -> bass.Bass:
    """SBUF-staged: load sup/qry once, fan out with broadcast store DMAs."""
    nc = bass.Bass("TRN2", target_bir_lowering=False)
    sup = nc.dram_tensor("sup", [NSUP, T, F], mybir.dt.float32, kind="ExternalInput")
    qry = nc.dram_tensor("qry", [NQ, T, F], mybir.dt.float32, kind="ExternalInput")
    out = nc.dram_tensor(
        "out", [NQ * NSUP, T, 2 * F], mybir.dt.float32, kind="ExternalOutput"
    )

    f32 = mybir.dt.float32
    with (
        nc.sbuf_tensor("supA", [128, NSUP * F], f32) as supA,
        nc.sbuf_tensor("supB", [128, NSUP * F], f32) as supB,
        nc.sbuf_tensor("qryA", [128, NQ * F], f32) as qryA,
        nc.sbuf_tensor("qryB", [128, NQ * F], f32) as qryB,
        nc.semaphore("ls_a") as ls_a,
        nc.semaphore("ls_b") as ls_b,
        nc.semaphore("lq_a") as lq_a,
        nc.semaphore("lq_b") as lq_b,
        nc.semaphore("done") as done,
        nc.Block() as block,
    ):
        sup_tiles = {0: supA, 1: supB}
        qry_tiles = {0: qryA, 1: qryB}
        sup_sems = {0: ls_a, 1: ls_b}
        qry_sems = {0: lq_a, 1: lq_b}

        # SBUF APs must keep the partition dim first; permute the DRAM dst
        # to the same (t, pair, f) iteration order instead.
        def sup_store(eng, il, h):
            # out[il*10:(il+1)*10, h*128:(h+1)*128, 0:F] = sup rows
            dst = out[il * NSUP : (il + 1) * NSUP, h * TH : (h + 1) * TH, 0:F]
            src = sup_tiles[h].ap().rearrange("p (j f) -> p j f", f=F)
            eng.dma_start(dst.transpose([1, 0, 2]), src).then_inc(done, 16)

        def qry_store(eng, il, h):
            dst = out[il * NSUP : (il + 1) * NSUP, h * TH : (h + 1) * TH, F : 2 * F]
            src = (
                qry_tiles[h]
                .ap()[:, il * F : (il + 1) * F]
                .unsqueeze(1)
                .broadcast_to([TH, NSUP, F])
            )
            eng.dma_start(dst.transpose([1, 0, 2]), src).then_inc(done, 16)

        @block.sync
        def _(sync):
            # loads: sup t-halves
            for h in (0, 1):
                sync.dma_start(
                    sup_tiles[h].ap(),
                    sup[:, h * TH : (h + 1) * TH, :].transpose([1, 0, 2]),
                ).then_inc(sup_sems[h], 16)
            for h in (0, 1):
                sync.wait_ge(sup_sems[h], 16)
                for il in range(NQ):
                    sup_store(sync, il, h)
            # everything must have landed before the NEFF may finish
            sync.wait_ge(done, 16 * 16)

        @block.scalar
        def _(scalar):
            for h in (0, 1):
                scalar.dma_start(
                    qry_tiles[h].ap(),
                    qry[:, h * TH : (h + 1) * TH, :].transpose([1, 0, 2]),
                ).then_inc(qry_sems[h], 16)
            for h in (0, 1):
                scalar.wait_ge(qry_sems[h], 16)
                for il in range(NQ):
                    qry_store(scalar, il, h)

    return nc


def _build_kernel_v3() -> bass.Bass:
    """DVE-assembled output rows in SBUF -> 4 KB DMA descriptors.

    SBUF partition p = t // 2 (each partition holds t rows 2p, 2p+1), so an
    assembled pair row block is contiguous for 4 KB in both SBUF free dim
    and the output: descriptors are 4x bigger than the raw 1 KB (f-run)
    limit, lifting the per-SDMA-engine descriptor-rate ceiling.

    Per query block il (10 pairs): staging S[p, (q, t2, f2)] free = 10*2*512.
      sup half:  S[p, q*1024 + t2*512 + f]       = sup[q, 2p+t2, f]
      qry half:  S[p, q*1024 + t2*512 + 256 + f] = qry[il, 2p+t2, f]
    One store DMA per (block, t2-pair granularity merged): dst flat block
    out[il*10:(il+1)*10] with AP [[1024,128],[131072,10],[1,1024]].
    """
    nc = bass.Bass("TRN2", target_bir_lowering=False)
    sup = nc.dram_tensor("sup", [NSUP, T, F], mybir.dt.float32, kind="ExternalInput")
    qry = nc.dram_tensor("qry", [NQ, T, F], mybir.dt.float32, kind="ExternalInput")
    out = nc.dram_tensor(
        "out", [NQ * NSUP, T, 2 * F], mybir.dt.float32, kind="ExternalOutput"
    )

    f32 = mybir.dt.float32
    NBUF = 2  # staging double buffer
    with (
        nc.sbuf_tensor("supt", [128, NSUP * 2 * F], f32) as supt,
        nc.sbuf_tensor("qryt", [128, NQ * 2 * F], f32) as qryt,
        nc.sbuf_tensor("stg0", [128, NSUP * 2 * 2 * F], f32) as stg0,
        nc.sbuf_tensor("stg1", [128, NSUP * 2 * 2 * F], f32) as stg1,
        nc.semaphore("load_sup") as load_sup,
        nc.semaphore("load_qry") as load_qry,
        nc.semaphore("dve_done") as dve_done,
        nc.semaphore("store_even") as store_even,
        nc.semaphore("store_odd") as store_odd,
        nc.Block() as block,
    ):
        stg = [stg0, stg1]
        store_sem = [store_even, store_odd]
        SW = NSUP * 2 * 2 * F  # staging free width (20480)

        def store_block(eng, il):
            eng.wait_ge(dve_done, 4 * (il + 1))
            dst = bass.AP(
                out,
                il * NSUP * T * 2 * F,
                [[2 * 2 * F, 128], [T * 2 * F, NSUP], [1, 2 * 2 * F]],
            )
            src = bass.AP(
                stg[il % NBUF], 0, [[SW, 128], [2 * 2 * F, NSUP], [1, 2 * 2 * F]]
            )
            eng.dma_start(dst, src).then_inc(store_sem[il % 2], 16)

        @block.sync
        def _(sync):
            # loads: input tiles with partition = t//2, free = (row, t2, f)
            for t2 in (0, 1):
                dst = supt.ap().rearrange("p (j t2 f) -> p j t2 f", t2=2, f=F)[
                    :, :, t2, :
                ]
                src = bass.AP(sup, t2 * F, [[2 * F, 128], [T * F, NSUP], [1, F]])
                sync.dma_start(dst, src).then_inc(load_sup, 16)
            for t2 in (0, 1):
                dst = qryt.ap().rearrange("p (i t2 f) -> p i t2 f", t2=2, f=F)[
                    :, :, t2, :
                ]
                src = bass.AP(qry, t2 * F, [[2 * F, 128], [T * F, NQ], [1, F]])
                sync.dma_start(dst, src).then_inc(load_qry, 16)
            for il in range(0, NQ, 2):
                store_block(sync, il)
            sync.wait_ge(store_even, 16 * (NQ // 2))
            sync.wait_ge(store_odd, 16 * (NQ // 2))

        @block.scalar
        def _(scalar):
            for il in range(1, NQ, 2):
                store_block(scalar, il)

        @block.vector
        def _(vector):
            vector.wait_ge(load_sup, 32)
            vector.wait_ge(load_qry, 32)
            for il in range(NQ):
                if il >= NBUF:
                    # staging slot reuse: wait for store of block il-NBUF
                    vector.wait_ge(store_sem[il % 2], 16 * ((il - NBUF) // 2 + 1))
                s = stg[il % NBUF]
                for t2 in (0, 1):
                    # sup half: S[p, q*1024 + t2*512 + f] = supt[p, q*512 + t2*256 + f]
                    dst = bass.AP(s, t2 * 2 * F, [[SW, 128], [2 * 2 * F, NSUP], [1, F]])
                    src = bass.AP(supt, t2 * F, [[NSUP * 2 * F, 128], [2 * F, NSUP], [1, F]])
                    vector.tensor_copy(dst, src).then_inc(dve_done, 1)
                    # qry half: S[..., +256] = qryt[p, il*512 + t2*256 + f] bcast over q
                    dstq = bass.AP(s, t2 * 2 * F + F, [[SW, 128], [2 * 2 * F, NSUP], [1, F]])
                    srcq = bass.AP(qryt, il * 2 * F + t2 * F, [[NQ * 2 * F, 128], [0, NSUP], [1, F]])
                    vector.tensor_copy(dstq, srcq).then_inc(dve_done, 1)

    return nc


def _build_kernel_v4() -> bass.Bass:
    """v3 + finer pipeline: 8 half-blocks (5 pairs) over 4 staging buffers,
    stores round-robined over 3 DMA queues (sync, scalar HWDGE + gpsimd
    SWDGE), loads split across queues, sup halves assembled once per buffer.
    """
    nc = bass.Bass("TRN2", target_bir_lowering=False)
    sup = nc.dram_tensor("sup", [NSUP, T, F], mybir.dt.float32, kind="ExternalInput")
    qry = nc.dram_tensor("qry", [NQ, T, F], mybir.dt.float32, kind="ExternalInput")
    out = nc.dram_tensor(
        "out", [NQ * NSUP, T, 2 * F], mybir.dt.float32, kind="ExternalOutput"
    )

    f32 = mybir.dt.float32
    HP = 5  # pairs per half-block
    NHB = (NQ * NSUP) // HP  # 8 half-blocks
    NBUF = 4
    SW = HP * 2 * 2 * F  # staging free width (5120)
    SUPW = NSUP * 2 * F  # supt free width
    QRYW = NQ * 2 * F
    with (
        nc.sbuf_tensor("supt", [128, SUPW], f32) as supt,
        nc.sbuf_tensor("qryt", [128, QRYW], f32) as qryt,
        nc.sbuf_tensor("hb0", [128, SW], f32) as hb0,
        nc.sbuf_tensor("hb1", [128, SW], f32) as hb1,
        nc.sbuf_tensor("hb2", [128, SW], f32) as hb2,
        nc.sbuf_tensor("hb3", [128, SW], f32) as hb3,
        nc.semaphore("load_sup") as load_sup,
        nc.semaphore("load_qry") as load_qry,
        nc.semaphore("dve_done") as dve_done,
        nc.semaphore("st_h0") as st_h0,
        nc.semaphore("st_h1") as st_h1,
        nc.semaphore("st_h2") as st_h2,
        nc.semaphore("st_h3") as st_h3,
        nc.semaphore("st_h4") as st_h4,
        nc.semaphore("st_h5") as st_h5,
        nc.semaphore("st_h6") as st_h6,
        nc.semaphore("st_h7") as st_h7,
        nc.Block() as block,
    ):
        hb = [hb0, hb1, hb2, hb3]
        # one sem per half-block store (SWDGE sems must start at 0)
        st_sem = [st_h0, st_h1, st_h2, st_h3, st_h4, st_h5, st_h6, st_h7]

        def load_sup_dma(eng, t2):
            dst = supt.ap().rearrange("p (j t2 f) -> p j t2 f", t2=2, f=F)[:, :, t2, :]
            src = bass.AP(sup, t2 * F, [[2 * F, 128], [T * F, NSUP], [1, F]])
            eng.dma_start(dst, src).then_inc(load_sup, 16)

        def load_qry_dma(eng, t2):
            dst = qryt.ap().rearrange("p (i t2 f) -> p i t2 f", t2=2, f=F)[:, :, t2, :]
            src = bass.AP(qry, t2 * F, [[2 * F, 128], [T * F, NQ], [1, F]])
            eng.dma_start(dst, src).then_inc(load_qry, 16)

        def store_half(eng, h):
            # store half-block h: wait for its dve copies (8 sup + 2*(h+1))
            eng.wait_ge(dve_done, 8 + 2 * (h + 1))
            dst = bass.AP(
                out,
                h * HP * T * 2 * F,
                [[2 * 2 * F, 128], [T * 2 * F, HP], [1, 2 * 2 * F]],
            )
            src = bass.AP(hb[h % NBUF], 0, [[SW, 128], [2 * 2 * F, HP], [1, 2 * 2 * F]])
            eng.dma_start(dst, src).then_inc(st_sem[h], 16)

        @block.sync
        def _(sync):
            load_sup_dma(sync, 0)
            for h in range(0, NHB, 3):
                store_half(sync, h)
            for h in range(NHB):
                sync.wait_ge(st_sem[h], 16)

        @block.scalar
        def _(scalar):
            load_sup_dma(scalar, 1)
            for h in range(1, NHB, 3):
                store_half(scalar, h)

        @block.gpsimd
        def _(gpsimd):
            load_qry_dma(gpsimd, 0)
            load_qry_dma(gpsimd, 1)
            for h in range(2, NHB, 3):
                store_half(gpsimd, h)

        @block.vector
        def _(vector):
            # sup halves, once per buffer: buffer k serves half-blocks
            # h = k (mod 4), whose j-range is (k % 2) * 5
            vector.wait_ge(load_sup, 32)
            for k in range(NBUF):
                j0 = (k % 2) * HP
                for t2 in (0, 1):
                    dst = bass.AP(hb[k], t2 * 2 * F, [[SW, 128], [2 * 2 * F, HP], [1, F]])
                    src = bass.AP(
                        supt, j0 * 2 * F + t2 * F, [[SUPW, 128], [2 * F, HP], [1, F]]
                    )
                    vector.tensor_copy(dst, src).then_inc(dve_done, 1)
            # qry halves per half-block
            vector.wait_ge(load_qry, 32)
            for h in range(NHB):
                if h >= NBUF:
                    # buffer h%NBUF freed once its previous store completed
                    vector.wait_ge(st_sem[h - NBUF], 16)
                il = h // 2
                for t2 in (0, 1):
                    dst = bass.AP(
                        hb[h % NBUF], t2 * 2 * F + F, [[SW, 128], [2 * 2 * F, HP], [1, F]]
                    )
                    src = bass.AP(
                        qryt, il * 2 * F + t2 * F, [[QRYW, 128], [0, HP], [1, F]]
                    )
                    vector.tensor_copy(dst, src).then_inc(dve_done, 1)

    return nc




def _build_kernel_v5() -> bass.Bass:
    """v4 refined: host passes t-major inputs (sup_t [T,10,F], qry_t [T,4,F])
    so each load is one DMA with 20/8 KB descriptors; DVE copies are ordered
    for the earliest possible first store; stores alternate across the two
    HWDGE queues (sync/scalar); gpsimd only carries no stores (SWDGE desc
    generation is too slow to add bandwidth beyond the ~480 GB/s ceiling).

    SBUF layouts (partition p = t // 2, t2 = t % 2):
      supt[p, t2*(10*F) + j*F + f] = sup_t[2p+t2, j, f]   (flat-contiguous load)
      qryt[p, t2*(4*F) + i*F + f]  = qry_t[2p+t2, i, f]
      staging hb[p, q*4F + t2*2F + f2] = out[5h+q, 2p+t2, f2]  (4 KB dst runs)
    """
    nc = bass.Bass("TRN2", target_bir_lowering=False)
    sup = nc.dram_tensor("sup", [T, NSUP, F], mybir.dt.float32, kind="ExternalInput")
    qry = nc.dram_tensor("qry", [T, NQ, F], mybir.dt.float32, kind="ExternalInput")
    out = nc.dram_tensor(
        "out", [NQ * NSUP, T, 2 * F], mybir.dt.float32, kind="ExternalOutput"
    )

    f32 = mybir.dt.float32
    HP = 5  # pairs per half-block
    NHB = (NQ * NSUP) // HP  # 8 half-blocks
    NBUF = 4
    SW = HP * 2 * 2 * F  # staging free width (5120)
    SUPW = NSUP * 2 * F  # 5120
    QRYW = NQ * 2 * F  # 2048
    with (
        nc.sbuf_tensor("supt", [128, SUPW], f32) as supt,
        nc.sbuf_tensor("qryt", [128, QRYW], f32) as qryt,
        nc.sbuf_tensor("hb0", [128, SW], f32) as hb0,
        nc.sbuf_tensor("hb1", [128, SW], f32) as hb1,
        nc.sbuf_tensor("hb2", [128, SW], f32) as hb2,
        nc.sbuf_tensor("hb3", [128, SW], f32) as hb3,
        nc.semaphore("load_sup") as load_sup,
        nc.semaphore("load_qry") as load_qry,
        nc.semaphore("dve_done") as dve_done,
        nc.semaphore("st_h0") as st_h0,
        nc.semaphore("st_h1") as st_h1,
        nc.semaphore("st_h2") as st_h2,
        nc.semaphore("st_h3") as st_h3,
        nc.semaphore("st_h4") as st_h4,
        nc.semaphore("st_h5") as st_h5,
        nc.semaphore("st_h6") as st_h6,
        nc.semaphore("st_h7") as st_h7,
        nc.Block() as block,
    ):
        hb = [hb0, hb1, hb2, hb3]
        st_sem = [st_h0, st_h1, st_h2, st_h3, st_h4, st_h5, st_h6, st_h7]
        # dve_done value at which half-block h is fully assembled:
        # order: supb0, qryh0, supb1, qryh1, supb2, qryh2, supb3, qryh3,
        #        qryh4..qryh7  (2 copies each step)
        ready = [4, 8, 12, 16, 18, 20, 22, 24]

        def store_half(eng, h):
            eng.wait_ge(dve_done, ready[h])
            dst = bass.AP(
                out,
                h * HP * T * 2 * F,
                [[2 * 2 * F, 128], [T * 2 * F, HP], [1, 2 * 2 * F]],
            )
            src = bass.AP(hb[h % NBUF], 0, [[SW, 128], [2 * 2 * F, HP], [1, 2 * 2 * F]])
            eng.dma_start(dst, src).then_inc(st_sem[h], 16)

        @block.sync
        def _(sync):
            # one flat-contiguous load: descriptors of SUPW floats (20 KB)
            sync.dma_start(
                supt.ap(), bass.AP(sup, 0, [[SUPW, 128], [1, SUPW]])
            ).then_inc(load_sup, 16)
            for h in range(0, NHB, 2):
                store_half(sync, h)
            for h in range(NHB):
                sync.wait_ge(st_sem[h], 16)

        @block.scalar
        def _(scalar):
            scalar.dma_start(
                qryt.ap(), bass.AP(qry, 0, [[QRYW, 128], [1, QRYW]])
            ).then_inc(load_qry, 16)
            for h in range(1, NHB, 2):
                store_half(scalar, h)

        @block.vector
        def _(vector):
            def sup_copy(k):
                j0 = (k % 2) * HP
                for t2 in (0, 1):
                    dst = bass.AP(hb[k], t2 * 2 * F, [[SW, 128], [4 * F, HP], [1, F]])
                    srcs = bass.AP(
                        supt,
                        t2 * (NSUP * F) + j0 * F,
                        [[SUPW, 128], [F, HP], [1, F]],
                    )
                    vector.tensor_copy(dst, srcs).then_inc(dve_done, 1)

            def qry_copy(h):
                il = h // 2
                for t2 in (0, 1):
                    dst = bass.AP(
                        hb[h % NBUF], t2 * 2 * F + F, [[SW, 128], [4 * F, HP], [1, F]]
                    )
                    srcq = bass.AP(
                        qryt, t2 * (NQ * F) + il * F, [[QRYW, 128], [0, HP], [1, F]]
                    )
                    vector.tensor_copy(dst, srcq).then_inc(dve_done, 1)

            vector.wait_ge(load_sup, 16)
            sup_copy(0)
            vector.wait_ge(load_qry, 16)
            qry_copy(0)
            for k in (1, 2, 3):
                sup_copy(k)
                qry_copy(k)
            for h in range(NBUF, NHB):
                vector.wait_ge(st_sem[h - NBUF], 16)
                qry_copy(h)

    return nc


def _get_nc() -> bass.Bass:
    if "nc" not in _CACHE:
        _CACHE["nc"] = _build_kernel_v5()
    return _CACHE["nc"]


def _make_in_maps(x: np.ndarray) -> list:
    x = np.asarray(x, dtype=np.float32)
    sup_t = np.ascontiguousarray(x[:NSUP].transpose(1, 0, 2))  # [256, 10, 256]
    queries = x[10:]  # [30, 256, 256]
    in_maps = []
    for c in range(N_CORES):
        q0 = QSTART[c]
        qry_t = np.ascontiguousarray(queries[q0 : q0 + NQ].transpose(1, 0, 2))
        in_maps.append({"sup": sup_t, "qry": qry_t})
    return in_maps


def kernel(x: np.ndarray) -> np.ndarray:
    in_maps = _make_in_maps(x)
    nc = _get_nc()
    res = run_bass_kernel_spmd(nc, in_maps, core_ids=list(range(N_CORES)))

    parts = []
    for c in range(N_CORES):
        lo, hi = KEEP[c]
        parts.append(res.results[c]["out"][lo:hi])
    full = np.concatenate(parts, axis=0)
    assert full.shape == (NQ_TOTAL * NSUP, T, 2 * F)
    return full
